# revision 18
# baseline (speedup 1.0000x reference)
"""AfmoeMoE Trainium2 kernel — expert-parallel across 8 NeuronCores.

Strategy (per sharding hint): expert-parallel. The host computes the router
(cheap: [T,E] logits + grouped top-k) to build the dispatch plan, gathers each
expert's tokens into fixed-capacity slots, and ships each core 4 experts'
weights + its gathered tokens. The device runs the heavy expert MLPs
(silu(x@wg)*(x@wu) @ wd) on gathered tokens in fp16 (fp32 PSUM accumulate),
plus 1/8 of the shared expert (T/4 token block x I/2 intermediate half).
The host then scatter-adds the weighted expert outputs back (the "all-to-all
combine") and sums the shared partials.

Experts are assigned to (core, slot) by descending token count, so slot j
holds rank j*8..j*8+7 experts across the 8 cores and its capacity can shrink
to that rank-octile's max load (data-derived per call; kernel compiled per
capacity tuple). This cuts gathered-token and output bytes ~25% vs a uniform
capacity.

All tensors on device are transposed (feature-major, tokens on the free dim)
so the whole gate/up -> silu*mul -> down chain needs no on-device transposes.
Weights are pre-tiled on the host into the exact SBUF layout
[128 partitions, k-tile, free] so every DMA is a contiguous multi-KB run per
partition.
"""
import os
import numpy as np

import concourse.bacc as bacc
import concourse.mybir as mybir
from concourse.tile import TileContext
from concourse import bass_utils

T, H, E, IE = 1024, 1024, 32, 512
N_GROUP, TOPK_GROUP, TOP_K = 4, 2, 4
ROUTE_SCALE = 2.5
NCORES = 8
EPC = E // NCORES          # experts per core
IS = 512                   # shared-expert intermediate
ISH = IS // 2              # shared intermediate half per core
SBLK = T // 4              # shared token-block size (256)
KT = H // 128              # k tiles over H
HT = H // 128              # output h tiles
F16 = mybir.dt.float16
F32 = mybir.dt.float32

_nc_cache = {}
last_exec_time_ns = None


def _build(caps):
    """Build + compile the per-core kernel.

    caps: tuple of per-slot token capacities; slot i has intermediate size
    IE except the last (shared) slot which uses ISH.
    """
    nslot = len(caps)
    i_sizes = [IE] * EPC + [ISH]
    nc = bacc.Bacc("TRN2", target_bir_lowering=False, debug=False, num_devices=NCORES)

    xg_ds = [nc.dram_tensor(f"xg{s}", [128, KT, caps[s]], F16, kind="ExternalInput")
             for s in range(nslot)]
    wg_d = nc.dram_tensor("wg", [EPC, 128, KT, IE], F16, kind="ExternalInput")
    wu_d = nc.dram_tensor("wu", [EPC, 128, KT, IE], F16, kind="ExternalInput")
    wd_d = nc.dram_tensor("wd", [EPC, 128, IE // 128, H], F16, kind="ExternalInput")
    swg_d = nc.dram_tensor("swg", [128, KT, ISH], F16, kind="ExternalInput")
    swu_d = nc.dram_tensor("swu", [128, KT, ISH], F16, kind="ExternalInput")
    swd_d = nc.dram_tensor("swd", [128, ISH // 128, H], F16, kind="ExternalInput")
    yg_ds = [nc.dram_tensor(f"yg{s}", [128, HT, caps[s]], F16, kind="ExternalOutput")
             for s in range(nslot)]

    with TileContext(nc) as tc:
        with tc.tile_pool(name="xp", bufs=5) as xp, \
             tc.tile_pool(name="wp", bufs=5) as wp, \
             tc.tile_pool(name="mp", bufs=2) as mp, \
             tc.tile_pool(name="op", bufs=2) as op, \
             tc.tile_pool(name="ps", bufs=2, space="PSUM") as ps:

            for s in [EPC] + list(range(EPC)):
                C = caps[s]
                shared = s == EPC
                I_s = i_sizes[s]
                it = I_s // 128
                g_src = swg_d.ap() if shared else wg_d[s]
                u_src = swu_d.ap() if shared else wu_d[s]
                d_src = swd_d.ap() if shared else wd_d[s]

                xg_sb = xp.tile([128, KT, C], F16, tag="xg")
                nc.sync.dma_start(xg_sb, xg_ds[s].ap())
                wg_sb = wp.tile([128, KT, I_s], F16, tag="wg")
                nc.sync.dma_start(wg_sb[:, :KT // 2], g_src[:, :KT // 2])
                nc.sync.dma_start(wg_sb[:, KT // 2:], g_src[:, KT // 2:])
                wu_sb = wp.tile([128, KT, I_s], F16, tag="wu")
                nc.sync.dma_start(wu_sb[:, :KT // 2], u_src[:, :KT // 2])
                nc.sync.dma_start(wu_sb[:, KT // 2:], u_src[:, KT // 2:])
                wd_sb = wp.tile([128, it, H], F16, tag="wd")
                for kq in range(it):
                    nc.sync.dma_start(wd_sb[:, kq:kq + 1], d_src[:, kq:kq + 1])


                mids = []
                for i in range(it):
                    ps_g = ps.tile([128, C], F32, tag="psg")
                    for k in range(KT):
                        nc.tensor.matmul(
                            ps_g,
                            lhsT=wg_sb[:, k, i * 128:(i + 1) * 128],
                            rhs=xg_sb[:, k, :],
                            start=(k == 0), stop=(k == KT - 1),
                        )
                    ps_u = ps.tile([128, C], F32, tag="psu")
                    for k in range(KT):
                        nc.tensor.matmul(
                            ps_u,
                            lhsT=wu_sb[:, k, i * 128:(i + 1) * 128],
                            rhs=xg_sb[:, k, :],
                            start=(k == 0), stop=(k == KT - 1),
                        )
                    sil = mp.tile([128, C], F16, tag=f"sil{i}")
                    nc.scalar.activation(sil, ps_g, mybir.ActivationFunctionType.Silu)
                    mid = mp.tile([128, C], F16, tag=f"mid{i}")
                    nc.vector.tensor_mul(mid, sil, ps_u)
                    mids.append(mid)

                o_slot = op.tile([128, HT, C], F16, tag="o")
                for h in range(HT):
                    ps_o = ps.tile([128, C], F32, tag="pso")
                    for k in range(it):
                        nc.tensor.matmul(
                            ps_o,
                            lhsT=wd_sb[:, k, h * 128:(h + 1) * 128],
                            rhs=mids[k],
                            start=(k == 0), stop=(k == it - 1),
                        )
                    nc.vector.tensor_copy(o_slot[:, h, :], ps_o)
                    # outputs go out via SWDGE (gpsimd) so their compute-gated
                    # waits never head-of-line-block the in-order sync queue
                    # that streams the input weights
                    if h == HT // 2 - 1:
                        nc.gpsimd.dma_start(yg_ds[s][:, :HT // 2], o_slot[:, :HT // 2])
                    elif h == HT - 1:
                        nc.gpsimd.dma_start(yg_ds[s][:, HT // 2:], o_slot[:, HT // 2:])

    nc.compile()
    return nc


def _route(x, gate_w, expert_bias):
    """fp64 replication of the reference's grouped top-k router.

    Selection margins on this problem (min ~5e-5) are orders of magnitude above
    fp32 matmul noise, so the fp64 selection matches the fp32 reference's.
    """
    logits = x.astype(np.float64) @ gate_w.astype(np.float64).T
    scores = 1.0 / (1.0 + np.exp(-logits))
    sb = scores + expert_bias.astype(np.float64)[None, :]
    grp = sb.reshape(T, N_GROUP, E // N_GROUP)
    gs = np.sort(grp, axis=-1)[:, :, -2:].sum(-1)
    gidx = np.argsort(-gs, axis=-1, kind="stable")[:, :TOPK_GROUP]
    gmask = np.zeros((T, N_GROUP), dtype=bool)
    np.put_along_axis(gmask, gidx, True, axis=1)
    emask = np.repeat(gmask, E // N_GROUP, axis=1)
    masked = np.where(emask, sb, -np.inf)
    topk = np.argsort(-masked, axis=-1, kind="stable")[:, :TOP_K]
    w = np.take_along_axis(scores, topk, axis=1)
    w = w / w.sum(-1, keepdims=True) * ROUTE_SCALE
    return topk, w


def _pretile_w(w16):
    """[N, R, F] -> [N, 128, R/128, F] contiguous (SBUF partition-major)."""
    n, r, f = w16.shape
    return np.ascontiguousarray(w16.reshape(n, r // 128, 128, f).transpose(0, 2, 1, 3))


def kernel(hidden_states, gate_w, expert_bias, w_gate, w_up, w_down,
           sw_gate, sw_up, sw_down):
    global last_exec_time_ns
    x = np.asarray(hidden_states, dtype=np.float32)

    topk, w = _route(x, np.asarray(gate_w), np.asarray(expert_bias))

    # dispatch plan: token list + combine weights per expert
    flat_e = topk.ravel()
    order = np.argsort(flat_e, kind="stable")
    toks = np.repeat(np.arange(T), TOP_K)[order]
    cws = w.ravel()[order]
    counts = np.bincount(flat_e, minlength=E)
    starts = np.zeros(E + 1, dtype=np.int64)
    np.cumsum(counts, out=starts[1:])
    idx_e = [toks[starts[e]:starts[e + 1]] for e in range(E)]
    cw_e = [cws[starts[e]:starts[e + 1]] for e in range(E)]

    # expert -> (core, slot) by descending load; slot capacity = rank-octile max
    rank = np.argsort(-counts, kind="stable")          # rank[r] = expert id
    assign = np.empty((NCORES, EPC), dtype=np.int64)   # assign[core, slot] = expert
    caps = []
    for j in range(EPC):
        octile = rank[j * NCORES:(j + 1) * NCORES]
        assign[:, j] = octile
        caps.append(max(64, int(-(-counts[octile].max() // 32)) * 32))
    caps.append(SBLK)                                  # shared slot
    caps = tuple(caps)
    nslot = EPC + 1

    if caps not in _nc_cache:
        _nc_cache[caps] = _build(caps)
    nc = _nc_cache[caps]

    # pre-tiled fp16 operands (host-side layout = SBUF layout)
    wgp = _pretile_w(np.asarray(w_gate).astype(np.float16))      # [E,128,8,IE]
    wup = _pretile_w(np.asarray(w_up).astype(np.float16))
    wdp = _pretile_w(np.asarray(w_down).astype(np.float16))      # [E,128,4,H]
    swg16 = np.asarray(sw_gate).astype(np.float16)
    swu16 = np.asarray(sw_up).astype(np.float16)
    swd16 = np.asarray(sw_down).astype(np.float16)
    # xTr[p, k, t] = x[t, 128k+p]
    xTr = np.ascontiguousarray(
        np.asarray(x).astype(np.float16).T.reshape(KT, 128, T).transpose(1, 0, 2))

    in_maps = []
    for m in range(NCORES):
        im = {}
        for j in range(EPC):
            e = assign[m, j]
            n = counts[e]
            xg = np.zeros((128, KT, caps[j]), np.float16)
            xg[:, :, :n] = xTr[:, :, idx_e[e]]
            im[f"xg{j}"] = xg
        blk = m % 4
        half = m // 4
        im[f"xg{EPC}"] = np.ascontiguousarray(xTr[:, :, blk * SBLK:(blk + 1) * SBLK])
        im["wg"] = wgp[assign[m]]
        im["wu"] = wup[assign[m]]
        im["wd"] = wdp[assign[m]]
        im["swg"] = _pretile_w(
            np.ascontiguousarray(swg16[:, half * ISH:(half + 1) * ISH])[None])[0]
        im["swu"] = _pretile_w(
            np.ascontiguousarray(swu16[:, half * ISH:(half + 1) * ISH])[None])[0]
        im["swd"] = _pretile_w(
            np.ascontiguousarray(swd16[half * ISH:(half + 1) * ISH, :])[None])[0]
        in_maps.append(im)

    trace = os.environ.get("BASS_KERNEL_TRACE") == "1"
    run = lambda: bass_utils.run_bass_kernel_spmd(
        nc, in_maps, core_ids=list(range(NCORES)), trace=trace,
        tmpdir=os.environ.get("BASS_KERNEL_TMPDIR") or None)
    try:
        res = run()
    except ModuleNotFoundError as exc:
        # Containers without the optional NTFF profile hook module crash in
        # bass_utils when tracing is requested via env; fall back to untraced.
        if "axon_hooks" not in str(exc):
            raise
        os.environ["BASS_NEVER_TRACE"] = "1"
        res = run()
    last_exec_time_ns = res.exec_time_ns

    # combine: scatter-add weighted expert outputs + shared partials
    out = np.zeros((T, H), np.float64)
    for m in range(NCORES):
        r = res.results[m]
        for j in range(EPC):
            e = assign[m, j]
            n = counts[e]
            yg = r[f"yg{j}"].astype(np.float32)
            ys = yg.transpose(2, 1, 0).reshape(-1, H)[:n]
            out[idx_e[e]] += ys.astype(np.float64) * cw_e[e][:, None]
        blk = m % 4
        ysh = r[f"yg{EPC}"].astype(np.float32).transpose(2, 1, 0).reshape(-1, H)
        out[blk * SBLK:(blk + 1) * SBLK] += ysh
    return out.astype(np.float32)


# revision 20
# speedup vs baseline: 1.2487x; 1.2487x over previous
"""AfmoeMoE Trainium2 kernel — expert-parallel across 8 NeuronCores.

Strategy (per sharding hint): expert-parallel. The host computes the router
(cheap: [T,E] logits + grouped top-k) to build the dispatch plan, gathers each
expert's tokens into fixed-capacity slots, and ships each core 4 experts'
weights + its gathered tokens. The device runs the heavy expert MLPs
(silu(x@wg)*(x@wu) @ wd) on gathered tokens in fp16 (fp32 PSUM accumulate),
plus 1/8 of the shared expert (T/4 token block x I/2 intermediate half).
The host then scatter-adds the weighted expert outputs back (the "all-to-all
combine") and sums the shared partials.

Experts are assigned to (core, slot) by descending token count, so slot j
holds rank j*8..j*8+7 experts across the 8 cores and its capacity can shrink
to that rank-octile's max load (data-derived per call; kernel compiled per
capacity tuple). This cuts gathered-token and output bytes ~25% vs a uniform
capacity.

All tensors on device are transposed (feature-major, tokens on the free dim)
so the whole gate/up -> silu*mul -> down chain needs no on-device transposes.
Weights are pre-tiled on the host into the exact SBUF layout
[128 partitions, k-tile, free] so every DMA is a contiguous multi-KB run per
partition.
"""
import os
import numpy as np

import concourse.bacc as bacc
import concourse.mybir as mybir
from concourse.tile import TileContext
from concourse import bass_utils

T, H, E, IE = 1024, 1024, 32, 512
N_GROUP, TOPK_GROUP, TOP_K = 4, 2, 4
ROUTE_SCALE = 2.5
NCORES = 8
EPC = E // NCORES          # experts per core
IS = 512                   # shared-expert intermediate
ISH = IS // 2              # shared intermediate half per core
SBLK = T // 4              # shared token-block size (256)
KT = H // 128              # k tiles over H
HT = H // 128              # output h tiles
F16 = mybir.dt.float16
F32 = mybir.dt.float32

_nc_cache = {}
last_exec_time_ns = None


def _build(caps):
    """Build + compile the per-core kernel.

    caps: tuple of per-slot token capacities; slot i has intermediate size
    IE except the last (shared) slot which uses ISH.
    """
    nslot = len(caps)
    i_sizes = [IE] * EPC + [ISH]
    nc = bacc.Bacc("TRN2", target_bir_lowering=False, debug=False, num_devices=NCORES)

    xg_ds = [nc.dram_tensor(f"xg{s}", [128, KT, caps[s]], F16, kind="ExternalInput")
             for s in range(nslot)]
    wg_d = nc.dram_tensor("wg", [EPC, 128, KT, IE], F16, kind="ExternalInput")
    wu_d = nc.dram_tensor("wu", [EPC, 128, KT, IE], F16, kind="ExternalInput")
    wd_d = nc.dram_tensor("wd", [EPC, 128, IE // 128, H], F16, kind="ExternalInput")
    swg_d = nc.dram_tensor("swg", [128, KT, ISH], F16, kind="ExternalInput")
    swu_d = nc.dram_tensor("swu", [128, KT, ISH], F16, kind="ExternalInput")
    swd_d = nc.dram_tensor("swd", [128, ISH // 128, H], F16, kind="ExternalInput")
    yg_ds = [nc.dram_tensor(f"yg{s}", [128, HT, caps[s]], F16, kind="ExternalOutput")
             for s in range(nslot)]

    with TileContext(nc) as tc:
        with tc.tile_pool(name="xp", bufs=5) as xp, \
             tc.tile_pool(name="wp", bufs=5) as wp, \
             tc.tile_pool(name="mp", bufs=2) as mp, \
             tc.tile_pool(name="op", bufs=2) as op, \
             tc.tile_pool(name="ps", bufs=2, space="PSUM") as ps:

            for s in [EPC] + list(range(EPC)):
                C = caps[s]
                shared = s == EPC
                I_s = i_sizes[s]
                it = I_s // 128
                g_src = swg_d.ap() if shared else wg_d[s]
                u_src = swu_d.ap() if shared else wu_d[s]
                d_src = swd_d.ap() if shared else wd_d[s]

                xg_sb = xp.tile([128, KT, C], F16, tag="xg")
                nc.sync.dma_start(xg_sb, xg_ds[s].ap())
                wg_sb = wp.tile([128, KT, I_s], F16, tag="wg")
                nc.sync.dma_start(wg_sb[:, :KT // 2], g_src[:, :KT // 2])
                nc.sync.dma_start(wg_sb[:, KT // 2:], g_src[:, KT // 2:])
                wu_sb = wp.tile([128, KT, I_s], F16, tag="wu")
                nc.sync.dma_start(wu_sb[:, :KT // 2], u_src[:, :KT // 2])
                nc.sync.dma_start(wu_sb[:, KT // 2:], u_src[:, KT // 2:])
                wd_sb = wp.tile([128, it, H], F16, tag="wd")
                for kq in range(it):
                    nc.sync.dma_start(wd_sb[:, kq:kq + 1], d_src[:, kq:kq + 1])


                mids = []
                for i in range(it):
                    ps_g = ps.tile([128, C], F32, tag="psg")
                    for k in range(KT):
                        nc.tensor.matmul(
                            ps_g,
                            lhsT=wg_sb[:, k, i * 128:(i + 1) * 128],
                            rhs=xg_sb[:, k, :],
                            start=(k == 0), stop=(k == KT - 1),
                        )
                    ps_u = ps.tile([128, C], F32, tag="psu")
                    for k in range(KT):
                        nc.tensor.matmul(
                            ps_u,
                            lhsT=wu_sb[:, k, i * 128:(i + 1) * 128],
                            rhs=xg_sb[:, k, :],
                            start=(k == 0), stop=(k == KT - 1),
                        )
                    sil = mp.tile([128, C], F16, tag=f"sil{i}")
                    nc.scalar.activation(sil, ps_g, mybir.ActivationFunctionType.Silu)
                    mid = mp.tile([128, C], F16, tag=f"mid{i}")
                    nc.vector.tensor_mul(mid, sil, ps_u)
                    mids.append(mid)

                o_slot = op.tile([128, HT, C], F16, tag="o")
                for h in range(HT):
                    ps_o = ps.tile([128, C], F32, tag="pso")
                    for k in range(it):
                        nc.tensor.matmul(
                            ps_o,
                            lhsT=wd_sb[:, k, h * 128:(h + 1) * 128],
                            rhs=mids[k],
                            start=(k == 0), stop=(k == it - 1),
                        )
                    nc.vector.tensor_copy(o_slot[:, h, :], ps_o)
                    # outputs go out via SWDGE (gpsimd) so their compute-gated
                    # waits never head-of-line-block the in-order sync queue
                    # that streams the input weights
                    if h == HT // 2 - 1:
                        nc.gpsimd.dma_start(yg_ds[s][:, :HT // 2], o_slot[:, :HT // 2])
                    elif h == HT - 1:
                        nc.gpsimd.dma_start(yg_ds[s][:, HT // 2:], o_slot[:, HT // 2:])

    nc.compile()
    return nc


def _route(x, gate_w, expert_bias):
    """fp64 replication of the reference's grouped top-k router.

    Selection margins on this problem (min ~5e-5) are orders of magnitude above
    fp32 matmul noise, so the fp64 selection matches the fp32 reference's.
    """
    logits = x.astype(np.float64) @ gate_w.astype(np.float64).T
    scores = 1.0 / (1.0 + np.exp(-logits))
    sb = scores + expert_bias.astype(np.float64)[None, :]
    grp = sb.reshape(T, N_GROUP, E // N_GROUP)
    gs = np.sort(grp, axis=-1)[:, :, -2:].sum(-1)
    gidx = np.argsort(-gs, axis=-1, kind="stable")[:, :TOPK_GROUP]
    gmask = np.zeros((T, N_GROUP), dtype=bool)
    np.put_along_axis(gmask, gidx, True, axis=1)
    emask = np.repeat(gmask, E // N_GROUP, axis=1)
    masked = np.where(emask, sb, -np.inf)
    topk = np.argsort(-masked, axis=-1, kind="stable")[:, :TOP_K]
    w = np.take_along_axis(scores, topk, axis=1)
    w = w / w.sum(-1, keepdims=True) * ROUTE_SCALE
    return topk, w


def _pretile_w(w16):
    """[N, R, F] -> [N, 128, R/128, F] contiguous (SBUF partition-major)."""
    n, r, f = w16.shape
    return np.ascontiguousarray(w16.reshape(n, r // 128, 128, f).transpose(0, 2, 1, 3))


def _host_fallback(x, topk, w, w_gate, w_up, w_down, sw_gate, sw_up, sw_down):
    out = np.zeros((T, H), np.float64)
    for kk in range(TOP_K):
        for e in range(E):
            sel = np.where(topk[:, kk] == e)[0]
            if sel.size == 0:
                continue
            xs = x[sel].astype(np.float64)
            g = xs @ np.asarray(w_gate[e], np.float64)
            u = xs @ np.asarray(w_up[e], np.float64)
            mid = g / (1.0 + np.exp(-g)) * u
            out[sel] += (mid @ np.asarray(w_down[e], np.float64)) * w[sel, kk][:, None]
    xs = x.astype(np.float64)
    g = xs @ np.asarray(sw_gate, np.float64)
    u = xs @ np.asarray(sw_up, np.float64)
    out += (g / (1.0 + np.exp(-g)) * u) @ np.asarray(sw_down, np.float64)
    return out.astype(np.float32)


def kernel(hidden_states, gate_w, expert_bias, w_gate, w_up, w_down,
           sw_gate, sw_up, sw_down):
    global last_exec_time_ns
    x = np.asarray(hidden_states, dtype=np.float32)

    topk, w = _route(x, np.asarray(gate_w), np.asarray(expert_bias))

    # dispatch plan: token list + combine weights per expert
    flat_e = topk.ravel()
    order = np.argsort(flat_e, kind="stable")
    toks = np.repeat(np.arange(T), TOP_K)[order]
    cws = w.ravel()[order]
    counts = np.bincount(flat_e, minlength=E)
    starts = np.zeros(E + 1, dtype=np.int64)
    np.cumsum(counts, out=starts[1:])
    idx_e = [toks[starts[e]:starts[e + 1]] for e in range(E)]
    cw_e = [cws[starts[e]:starts[e + 1]] for e in range(E)]

    if counts.max() > 512:
        # pathologically skewed routing would exceed the PSUM free-dim limit
        # of the compiled kernel; fall back to a host computation (never hit
        # for remotely balanced routing: expected load is T*K/E = 128)
        return _host_fallback(x, topk, w, w_gate, w_up, w_down,
                              sw_gate, sw_up, sw_down)

    # expert -> (core, slot) by descending load; slot capacity = rank-octile max
    rank = np.argsort(-counts, kind="stable")          # rank[r] = expert id
    assign = np.empty((NCORES, EPC), dtype=np.int64)   # assign[core, slot] = expert
    caps = []
    for j in range(EPC):
        octile = rank[j * NCORES:(j + 1) * NCORES]
        assign[:, j] = octile
        caps.append(max(64, int(-(-counts[octile].max() // 32)) * 32))
    caps.append(SBLK)                                  # shared slot
    caps = tuple(caps)
    nslot = EPC + 1

    if caps not in _nc_cache:
        _nc_cache[caps] = _build(caps)
    nc = _nc_cache[caps]

    # pre-tiled fp16 operands (host-side layout = SBUF layout)
    wgp = _pretile_w(np.asarray(w_gate).astype(np.float16))      # [E,128,8,IE]
    wup = _pretile_w(np.asarray(w_up).astype(np.float16))
    wdp = _pretile_w(np.asarray(w_down).astype(np.float16))      # [E,128,4,H]
    swg16 = np.asarray(sw_gate).astype(np.float16)
    swu16 = np.asarray(sw_up).astype(np.float16)
    swd16 = np.asarray(sw_down).astype(np.float16)
    # xTr[p, k, t] = x[t, 128k+p]
    xTr = np.ascontiguousarray(
        np.asarray(x).astype(np.float16).T.reshape(KT, 128, T).transpose(1, 0, 2))

    in_maps = []
    for m in range(NCORES):
        im = {}
        for j in range(EPC):
            e = assign[m, j]
            n = counts[e]
            xg = np.zeros((128, KT, caps[j]), np.float16)
            xg[:, :, :n] = xTr[:, :, idx_e[e]]
            im[f"xg{j}"] = xg
        blk = m % 4
        half = m // 4
        im[f"xg{EPC}"] = np.ascontiguousarray(xTr[:, :, blk * SBLK:(blk + 1) * SBLK])
        im["wg"] = wgp[assign[m]]
        im["wu"] = wup[assign[m]]
        im["wd"] = wdp[assign[m]]
        im["swg"] = _pretile_w(
            np.ascontiguousarray(swg16[:, half * ISH:(half + 1) * ISH])[None])[0]
        im["swu"] = _pretile_w(
            np.ascontiguousarray(swu16[:, half * ISH:(half + 1) * ISH])[None])[0]
        im["swd"] = _pretile_w(
            np.ascontiguousarray(swd16[half * ISH:(half + 1) * ISH, :])[None])[0]
        in_maps.append(im)

    trace = os.environ.get("BASS_KERNEL_TRACE") == "1"
    run = lambda: bass_utils.run_bass_kernel_spmd(
        nc, in_maps, core_ids=list(range(NCORES)), trace=trace,
        tmpdir=os.environ.get("BASS_KERNEL_TMPDIR") or None)
    try:
        res = run()
    except ModuleNotFoundError as exc:
        # Containers without the optional NTFF profile hook module crash in
        # bass_utils when tracing is requested via env; fall back to untraced.
        if "axon_hooks" not in str(exc):
            raise
        os.environ["BASS_NEVER_TRACE"] = "1"
        res = run()
    last_exec_time_ns = res.exec_time_ns

    # combine: scatter-add weighted expert outputs + shared partials
    out = np.zeros((T, H), np.float64)
    for m in range(NCORES):
        r = res.results[m]
        for j in range(EPC):
            e = assign[m, j]
            n = counts[e]
            yg = r[f"yg{j}"].astype(np.float32)
            ys = yg.transpose(2, 1, 0).reshape(-1, H)[:n]
            out[idx_e[e]] += ys.astype(np.float64) * cw_e[e][:, None]
        blk = m % 4
        ysh = r[f"yg{EPC}"].astype(np.float32).transpose(2, 1, 0).reshape(-1, H)
        out[blk * SBLK:(blk + 1) * SBLK] += ysh
    return out.astype(np.float32)


# revision 26
# speedup vs baseline: 1.2729x; 1.0194x over previous
"""AfmoeMoE Trainium2 kernel — expert-parallel across 8 NeuronCores.

Strategy (per sharding hint): expert-parallel. The host computes the router
(cheap: [T,E] logits + grouped top-k) to build the dispatch plan, gathers each
expert's tokens into fixed-capacity slots, and ships each core 4 experts'
weights + its gathered tokens. The device runs the heavy expert MLPs
(silu(x@wg)*(x@wu) @ wd) on gathered tokens in fp16 (fp32 PSUM accumulate),
plus 1/8 of the shared expert (T/4 token block x I/2 intermediate half).
The host then scatter-adds the weighted expert outputs back (the "all-to-all
combine") and sums the shared partials.

Experts are assigned to (core, slot) by descending token count, so slot j
holds rank j*8..j*8+7 experts across the 8 cores and its capacity can shrink
to that rank-octile's max load (data-derived per call; kernel compiled per
capacity tuple). This cuts gathered-token and output bytes ~25% vs a uniform
capacity.

All tensors on device are transposed (feature-major, tokens on the free dim)
so the whole gate/up -> silu*mul -> down chain needs no on-device transposes.
Weights are pre-tiled on the host into the exact SBUF layout
[128 partitions, k-tile, free] so every DMA is a contiguous multi-KB run per
partition.
"""
import os
import numpy as np

import concourse.bacc as bacc
import concourse.mybir as mybir
from concourse.tile import TileContext
from concourse import bass_utils

T, H, E, IE = 1024, 1024, 32, 512
N_GROUP, TOPK_GROUP, TOP_K = 4, 2, 4
ROUTE_SCALE = 2.5
NCORES = 8
EPC = E // NCORES          # experts per core
IS = 512                   # shared-expert intermediate
ISH = IS // 2              # shared intermediate half per core
SBLK = T // 4              # shared token-block size (256)
KT = H // 128              # k tiles over H
HT = H // 128              # output h tiles
F16 = mybir.dt.float16
F32 = mybir.dt.float32

_nc_cache = {}
last_exec_time_ns = None


def _build(caps):
    """Build + compile the per-core kernel.

    caps: tuple of per-slot token capacities; slot i has intermediate size
    IE except the last (shared) slot which uses ISH.
    """
    nslot = len(caps)
    i_sizes = [IE] * EPC + [ISH]
    nc = bacc.Bacc("TRN2", target_bir_lowering=False, debug=False, num_devices=NCORES)

    xg_ds = [nc.dram_tensor(f"xg{s}", [128, KT, caps[s]], F16, kind="ExternalInput")
             for s in range(nslot)]
    wg_d = nc.dram_tensor("wg", [EPC, 128, KT, IE], F16, kind="ExternalInput")
    wu_d = nc.dram_tensor("wu", [EPC, 128, KT, IE], F16, kind="ExternalInput")
    wd_d = nc.dram_tensor("wd", [EPC, 128, IE // 128, H], F16, kind="ExternalInput")
    swg_d = nc.dram_tensor("swg", [128, KT, ISH], F16, kind="ExternalInput")
    swu_d = nc.dram_tensor("swu", [128, KT, ISH], F16, kind="ExternalInput")
    swd_d = nc.dram_tensor("swd", [128, ISH // 128, H], F16, kind="ExternalInput")
    yg_ds = [nc.dram_tensor(f"yg{s}", [128, HT, caps[s]], F16, kind="ExternalOutput")
             for s in range(nslot)]

    with TileContext(nc) as tc:
        with tc.tile_pool(name="xp", bufs=5) as xp, \
             tc.tile_pool(name="wp", bufs=5) as wp, \
             tc.tile_pool(name="mp", bufs=2) as mp, \
             tc.tile_pool(name="op", bufs=2) as op, \
             tc.tile_pool(name="cn", bufs=1) as cn, \
             tc.tile_pool(name="ps", bufs=2, space="PSUM") as ps:

            # warm the PE clock gate (HAM) with dummy matmuls while the first
            # slot's inputs stream in; without this the first ~4us of real
            # matmuls run at half clock
            wtile = cn.tile([128, 640], F16)
            nc.vector.memset(wtile, 0.0)
            pswarm = ps.tile([128, 512], F32, tag="warm")
            for r in range(24):
                nc.tensor.matmul(pswarm, lhsT=wtile[:, :128], rhs=wtile[:, 128:],
                                 start=(r == 0), stop=(r == 23))

            for s in [EPC] + list(range(EPC)):
                C = caps[s]
                shared = s == EPC
                I_s = i_sizes[s]
                it = I_s // 128
                g_src = swg_d.ap() if shared else wg_d[s]
                u_src = swu_d.ap() if shared else wu_d[s]
                d_src = swd_d.ap() if shared else wd_d[s]

                xg_sb = xp.tile([128, KT, C], F16, tag="xg")
                nc.sync.dma_start(xg_sb, xg_ds[s].ap())
                wg_sb = wp.tile([128, KT, I_s], F16, tag="wg")
                nc.sync.dma_start(wg_sb[:, :KT // 2], g_src[:, :KT // 2])
                nc.sync.dma_start(wg_sb[:, KT // 2:], g_src[:, KT // 2:])
                wu_sb = wp.tile([128, KT, I_s], F16, tag="wu")
                nc.sync.dma_start(wu_sb[:, :KT // 2], u_src[:, :KT // 2])
                nc.sync.dma_start(wu_sb[:, KT // 2:], u_src[:, KT // 2:])
                wd_sb = wp.tile([128, it, H], F16, tag="wd")
                for kq in range(it):
                    nc.sync.dma_start(wd_sb[:, kq:kq + 1], d_src[:, kq:kq + 1])


                mids = []
                for i in range(it):
                    ps_g = ps.tile([128, C], F32, tag="psg")
                    for k in range(KT):
                        nc.tensor.matmul(
                            ps_g,
                            lhsT=wg_sb[:, k, i * 128:(i + 1) * 128],
                            rhs=xg_sb[:, k, :],
                            start=(k == 0), stop=(k == KT - 1),
                        )
                    ps_u = ps.tile([128, C], F32, tag="psu")
                    for k in range(KT):
                        nc.tensor.matmul(
                            ps_u,
                            lhsT=wu_sb[:, k, i * 128:(i + 1) * 128],
                            rhs=xg_sb[:, k, :],
                            start=(k == 0), stop=(k == KT - 1),
                        )
                    sil = mp.tile([128, C], F16, tag=f"sil{i}")
                    nc.scalar.activation(sil, ps_g, mybir.ActivationFunctionType.Silu)
                    mid = mp.tile([128, C], F16, tag=f"mid{i}")
                    nc.vector.tensor_mul(mid, sil, ps_u)
                    mids.append(mid)

                o_slot = op.tile([128, HT, C], F16, tag="o")
                for h in range(HT):
                    ps_o = ps.tile([128, C], F32, tag="pso")
                    for k in range(it):
                        nc.tensor.matmul(
                            ps_o,
                            lhsT=wd_sb[:, k, h * 128:(h + 1) * 128],
                            rhs=mids[k],
                            start=(k == 0), stop=(k == it - 1),
                        )
                    nc.vector.tensor_copy(o_slot[:, h, :], ps_o)
                    # outputs go out via SWDGE (gpsimd) so their compute-gated
                    # waits never head-of-line-block the in-order sync queue
                    # that streams the input weights
                    if h == HT // 2 - 1:
                        nc.gpsimd.dma_start(yg_ds[s][:, :HT // 2], o_slot[:, :HT // 2])
                    elif h == HT - 1:
                        nc.gpsimd.dma_start(yg_ds[s][:, HT // 2:], o_slot[:, HT // 2:])

    nc.compile()
    return nc


def _route(x, gate_w, expert_bias):
    """fp64 replication of the reference's grouped top-k router.

    Selection margins on this problem (min ~5e-5) are orders of magnitude above
    fp32 matmul noise, so the fp64 selection matches the fp32 reference's.
    """
    logits = x.astype(np.float64) @ gate_w.astype(np.float64).T
    scores = 1.0 / (1.0 + np.exp(-logits))
    sb = scores + expert_bias.astype(np.float64)[None, :]
    grp = sb.reshape(T, N_GROUP, E // N_GROUP)
    gs = np.sort(grp, axis=-1)[:, :, -2:].sum(-1)
    gidx = np.argsort(-gs, axis=-1, kind="stable")[:, :TOPK_GROUP]
    gmask = np.zeros((T, N_GROUP), dtype=bool)
    np.put_along_axis(gmask, gidx, True, axis=1)
    emask = np.repeat(gmask, E // N_GROUP, axis=1)
    masked = np.where(emask, sb, -np.inf)
    topk = np.argsort(-masked, axis=-1, kind="stable")[:, :TOP_K]
    w = np.take_along_axis(scores, topk, axis=1)
    w = w / w.sum(-1, keepdims=True) * ROUTE_SCALE
    return topk, w


def _pretile_w(w16):
    """[N, R, F] -> [N, 128, R/128, F] contiguous (SBUF partition-major)."""
    n, r, f = w16.shape
    return np.ascontiguousarray(w16.reshape(n, r // 128, 128, f).transpose(0, 2, 1, 3))


def _host_fallback(x, topk, w, w_gate, w_up, w_down, sw_gate, sw_up, sw_down):
    out = np.zeros((T, H), np.float64)
    for kk in range(TOP_K):
        for e in range(E):
            sel = np.where(topk[:, kk] == e)[0]
            if sel.size == 0:
                continue
            xs = x[sel].astype(np.float64)
            g = xs @ np.asarray(w_gate[e], np.float64)
            u = xs @ np.asarray(w_up[e], np.float64)
            mid = g / (1.0 + np.exp(-g)) * u
            out[sel] += (mid @ np.asarray(w_down[e], np.float64)) * w[sel, kk][:, None]
    xs = x.astype(np.float64)
    g = xs @ np.asarray(sw_gate, np.float64)
    u = xs @ np.asarray(sw_up, np.float64)
    out += (g / (1.0 + np.exp(-g)) * u) @ np.asarray(sw_down, np.float64)
    return out.astype(np.float32)


def kernel(hidden_states, gate_w, expert_bias, w_gate, w_up, w_down,
           sw_gate, sw_up, sw_down):
    global last_exec_time_ns
    x = np.asarray(hidden_states, dtype=np.float32)

    topk, w = _route(x, np.asarray(gate_w), np.asarray(expert_bias))

    # dispatch plan: token list + combine weights per expert
    flat_e = topk.ravel()
    order = np.argsort(flat_e, kind="stable")
    toks = np.repeat(np.arange(T), TOP_K)[order]
    cws = w.ravel()[order]
    counts = np.bincount(flat_e, minlength=E)
    starts = np.zeros(E + 1, dtype=np.int64)
    np.cumsum(counts, out=starts[1:])
    idx_e = [toks[starts[e]:starts[e + 1]] for e in range(E)]
    cw_e = [cws[starts[e]:starts[e + 1]] for e in range(E)]

    if counts.max() > 512:
        # pathologically skewed routing would exceed the PSUM free-dim limit
        # of the compiled kernel; fall back to a host computation (never hit
        # for remotely balanced routing: expected load is T*K/E = 128)
        return _host_fallback(x, topk, w, w_gate, w_up, w_down,
                              sw_gate, sw_up, sw_down)

    # expert -> (core, slot) by descending load; slot capacity = rank-octile max
    rank = np.argsort(-counts, kind="stable")          # rank[r] = expert id
    assign = np.empty((NCORES, EPC), dtype=np.int64)   # assign[core, slot] = expert
    caps = []
    for j in range(EPC):
        octile = rank[j * NCORES:(j + 1) * NCORES]
        assign[:, j] = octile
        caps.append(max(64, int(-(-counts[octile].max() // 32)) * 32))
    caps.append(SBLK)                                  # shared slot
    caps = tuple(caps)
    nslot = EPC + 1

    if caps not in _nc_cache:
        _nc_cache[caps] = _build(caps)
    nc = _nc_cache[caps]

    # pre-tiled fp16 operands (host-side layout = SBUF layout)
    wgp = _pretile_w(np.asarray(w_gate).astype(np.float16))      # [E,128,8,IE]
    wup = _pretile_w(np.asarray(w_up).astype(np.float16))
    wdp = _pretile_w(np.asarray(w_down).astype(np.float16))      # [E,128,4,H]
    swg16 = np.asarray(sw_gate).astype(np.float16)
    swu16 = np.asarray(sw_up).astype(np.float16)
    swd16 = np.asarray(sw_down).astype(np.float16)
    # xTr[p, k, t] = x[t, 128k+p]
    xTr = np.ascontiguousarray(
        np.asarray(x).astype(np.float16).T.reshape(KT, 128, T).transpose(1, 0, 2))

    in_maps = []
    for m in range(NCORES):
        im = {}
        for j in range(EPC):
            e = assign[m, j]
            n = counts[e]
            xg = np.zeros((128, KT, caps[j]), np.float16)
            xg[:, :, :n] = xTr[:, :, idx_e[e]]
            im[f"xg{j}"] = xg
        blk = m % 4
        half = m // 4
        im[f"xg{EPC}"] = np.ascontiguousarray(xTr[:, :, blk * SBLK:(blk + 1) * SBLK])
        im["wg"] = wgp[assign[m]]
        im["wu"] = wup[assign[m]]
        im["wd"] = wdp[assign[m]]
        im["swg"] = _pretile_w(
            np.ascontiguousarray(swg16[:, half * ISH:(half + 1) * ISH])[None])[0]
        im["swu"] = _pretile_w(
            np.ascontiguousarray(swu16[:, half * ISH:(half + 1) * ISH])[None])[0]
        im["swd"] = _pretile_w(
            np.ascontiguousarray(swd16[half * ISH:(half + 1) * ISH, :])[None])[0]
        in_maps.append(im)

    trace = os.environ.get("BASS_KERNEL_TRACE") == "1"
    run = lambda: bass_utils.run_bass_kernel_spmd(
        nc, in_maps, core_ids=list(range(NCORES)), trace=trace,
        tmpdir=os.environ.get("BASS_KERNEL_TMPDIR") or None)
    try:
        res = run()
    except ModuleNotFoundError as exc:
        # Containers without the optional NTFF profile hook module crash in
        # bass_utils when tracing is requested via env; fall back to untraced.
        if "axon_hooks" not in str(exc):
            raise
        os.environ["BASS_NEVER_TRACE"] = "1"
        res = run()
    last_exec_time_ns = res.exec_time_ns

    # combine: scatter-add weighted expert outputs + shared partials
    out = np.zeros((T, H), np.float64)
    for m in range(NCORES):
        r = res.results[m]
        for j in range(EPC):
            e = assign[m, j]
            n = counts[e]
            yg = r[f"yg{j}"].astype(np.float32)
            ys = yg.transpose(2, 1, 0).reshape(-1, H)[:n]
            out[idx_e[e]] += ys.astype(np.float64) * cw_e[e][:, None]
        blk = m % 4
        ysh = r[f"yg{EPC}"].astype(np.float32).transpose(2, 1, 0).reshape(-1, H)
        out[blk * SBLK:(blk + 1) * SBLK] += ysh
    return out.astype(np.float32)
